# revision 1
# baseline (speedup 1.0000x reference)
"""RNN-T decoder + joint network Trainium2 kernel (8 cores, data-parallel
over batch B=16 -> 2 per core; full inputs in, full output out).

Host side: embedding gather (indexing only), layout transposes, bf16 casts,
gate reorder i,f,g,o -> i,f,o,g, final bf16->fp32 upcast of the output.

Device side, per core, feature-major (partition = feature):
  * enc_pT = W_enc @ hsT + b_enc (bf16), xp0T = W_ih0 @ eysT + b0 as
    batched GEMMs (input projections hoisted out of the recurrence).
  * LSTM: W_hh matmuls -> PSUM gates; xp added by VectorE; sigmoid over
    i,f,o + tanh over g on ScalarE; state updates on VectorE.
  * Joint restructured for engine balance (CoreSim cost model):
      - DVE pre-add (4x mode, 112ns/200col): zt = encp_bf16 + decp[u]
      - ACT big-tile in-place tanh [128,1600] (1518ns for 8 u-cols)
      - PE W_out matmuls into 2-bank "duo" PSUM tiles [128,1024]
      - Pool (gpsimd) drains PSUM->obs bf16 with fused b_out (667ns/duo)
      - SP flushes obs [128,8u,200t] bf16 to DRAM (HWDGE)
    The joint is staggered one half-block: matmuls for (b1, blk-1) run
    while (b0, blk) pre-adds/tanhs, so PE never waits on prep.
  * Output DRAM tensor is bf16 (halves DMA_ENGINES time); host upcasts.
  * Input DMAs ordered so first compute (xp0: wih0+eyst; enc: wenc+hst)
    starts ~8us in instead of waiting for all weights.
"""

import os
import sys

import numpy as np

sys.path.insert(0, "/opt/trn_rl_repo")

import ml_dtypes  # noqa: E402
import concourse.bass as bass  # noqa: E402
from concourse import bacc  # noqa: E402
import concourse.mybir as mybir  # noqa: E402
import concourse.tile as tile  # noqa: E402
from concourse.bass_utils import run_bass_kernel_spmd  # noqa: E402

F32 = mybir.dt.float32
BF16 = mybir.dt.bfloat16
AF = mybir.ActivationFunctionType
ALU = mybir.AluOpType
BF_NP = ml_dtypes.bfloat16

NCORES = 8
B = 2        # batch per core
T = 200
U = 64
E = 512      # encoder proj dim
D = 512      # decoder hidden
J = 512      # joint dim
ODIM = 600
KB = 4       # 512 // 128
GT = 16      # 2048 // 128 gate tiles
R = B * U    # 128 LSTM rows per core
RT = B * T   # 400 encoder rows per core
OMW = [128, 128, 128, 128, 88]  # output feature tiles (600)
NM = len(OMW)
UB = 8       # u's per pipeline block
NBLK = U // UB


def _build():
    nc = bacc.Bacc()

    hst = nc.dram_tensor("hst", [E, RT], BF16, kind="ExternalInput")
    eyst = nc.dram_tensor("eyst", [E, R], BF16, kind="ExternalInput")
    wih0t = nc.dram_tensor("wih0t", [E, 4 * D], BF16, kind="ExternalInput")
    whh0t = nc.dram_tensor("whh0t", [D, 4 * D], BF16, kind="ExternalInput")
    wih1t = nc.dram_tensor("wih1t", [D, 4 * D], BF16, kind="ExternalInput")
    whh1t = nc.dram_tensor("whh1t", [D, 4 * D], BF16, kind="ExternalInput")
    wenct = nc.dram_tensor("wenct", [E, J], BF16, kind="ExternalInput")
    wdect = nc.dram_tensor("wdect", [D, J], BF16, kind="ExternalInput")
    woutt = nc.dram_tensor("woutt", [J, ODIM], BF16, kind="ExternalInput")
    bias0 = nc.dram_tensor("bias0", [128, GT], F32, kind="ExternalInput")
    b1w = nc.dram_tensor("b1w", [1, 4 * D], BF16, kind="ExternalInput")
    benc = nc.dram_tensor("benc", [128, KB], F32, kind="ExternalInput")
    bout = nc.dram_tensor("bout", [128, NM], F32, kind="ExternalInput")
    outt = nc.dram_tensor("outt", [B, ODIM, U, T], BF16, kind="ExternalOutput")

    with tile.TileContext(nc) as tc:
        with (
            tc.tile_pool(name="const", bufs=1) as cp,
            tc.tile_pool(name="work", bufs=2) as wp,
            tc.tile_pool(name="zt", bufs=5) as zp,
            tc.tile_pool(name="osb", bufs=11) as obp,
            tc.tile_pool(name="ps", bufs=1, space="PSUM") as psp,
            tc.tile_pool(name="pg", bufs=1, space="PSUM") as pgp,
        ):
            def load_kt(dram, cols, name):
                ts_ = []
                for k in range(dram.shape[0] // 128):
                    t = cp.tile([128, cols], BF16, tag=f"{name}{k}")
                    nc.sync.dma_start(out=t[:], in_=dram[k * 128:(k + 1) * 128, :])
                    ts_.append(t)
                return ts_

            def load_bias(dram, cols, name):
                raw = cp.tile([128, cols], F32, tag=f"{name}_raw")
                nc.sync.dma_start(out=raw[:], in_=dram[:, :])
                stg = cp.tile([128, cols], F32, tag=name)
                nc.vector.tensor_copy(stg[:], raw[:])  # stage onto DVE
                return stg

            # load order: first-needed first, k-tiles round-robin so the
            # k=0 accumulation chains can start after ~1/4 of the bytes
            # (xp0 <- wih0+eyst, enc <- wenc+hst, L0 <- whh0; wout last)
            def load_rr(specs):
                outs = [[] for _ in specs]
                for k in range(KB):
                    for si, (dram, cols, name) in enumerate(specs):
                        t = cp.tile([128, cols], BF16, tag=f"{name}{k}")
                        nc.sync.dma_start(
                            out=t[:], in_=dram[k * 128:(k + 1) * 128, :])
                        outs[si].append(t)
                return outs

            wih0_sb, eyst_sb, wenc_sb, hst_sb = load_rr([
                (wih0t, 4 * D, "wih0"), (eyst, R, "eyst"),
                (wenct, J, "wenc"), (hst, RT, "hst")])
            whh0_sb = load_kt(whh0t, 4 * D, "whh0")
            b0_sb = load_bias(bias0, GT, "b0")
            b1w_sb = cp.tile([1, 4 * D], BF16, tag="b1w")
            nc.sync.dma_start(out=b1w_sb[:], in_=b1w[:, :])
            ones_sb = cp.tile([1, B * UB], BF16, tag="ones")
            nc.vector.memset(ones_sb[:], 1.0)
            benc_sb = load_bias(benc, KB, "benc")
            bout_sb = load_bias(bout, NM, "bout")
            wih1_sb = load_kt(wih1t, 4 * D, "wih1")
            whh1_sb = load_kt(whh1t, 4 * D, "whh1")
            wdec_sb = load_kt(wdect, J, "wdec")
            wout_sb = load_kt(woutt, ODIM, "wout")

            # persistent state / intermediates
            c0 = cp.tile([128, KB * B], F32, tag="c0")
            c1 = cp.tile([128, KB * B], F32, tag="c1")
            h0all = cp.tile([128, KB * R], BF16, tag="h0all")  # col k*128+b*64+u
            h1all = cp.tile([128, KB * R], BF16, tag="h1all")
            xp0 = cp.tile([128, GT * R], F32, tag="xp0")  # col t*128+b*64+u
            xp1 = cp.tile([128, GT * R], F32, tag="xp1")
            decp = cp.tile([128, KB * R], F32, tag="decp")  # col m*128+b*64+u
            encp = cp.tile([128, KB * RT], BF16, tag="encp")  # col m*400+b*200+t

            nc.vector.memset(c0[:], 0.0)
            nc.vector.memset(c1[:], 0.0)

            POOLS = {}

            # ---- encoder projection: enc_pT = W_enc @ hsT + b_enc (bf16) ----
            def enc_proj(m):
                pe_ = POOLS["ps0"].tile([128, RT], F32, tag="ps")
                for k in range(KB):
                    nc.tensor.matmul(
                        pe_[:], wenc_sb[k][:, m * 128:(m + 1) * 128], hst_sb[k][:],
                        start=(k == 0), stop=(k == KB - 1))
                nc.vector.tensor_scalar_add(
                    encp[:, m * RT:(m + 1) * RT], pe_[:], benc_sb[:, m:m + 1])

            # ---- xp = W_ih @ rhs + bias (batched input projections) ----
            def in_proj(w_sb, rhs_fn, bias_sb, dst):
                for t in range(GT):
                    pb = POOLS["ps0"].tile([128, R], F32, tag="ps")
                    for k in range(KB):
                        nc.tensor.matmul(
                            pb[:], w_sb[k][:, t * 128:(t + 1) * 128], rhs_fn(k),
                            start=(k == 0), stop=(k == KB - 1))
                    nc.vector.tensor_scalar_add(
                        dst[:, t * R:(t + 1) * R], pb[:], bias_sb[:, t:t + 1])

            # gate order (host-permuted): i, f, o, g
            def lstm_step(u, xp, whh_sb, cst, hall):
                hav = hall[:].rearrange("p (k b u) -> p k b u", k=KB, b=B)
                xpv = xp[:].rearrange("p (t b u) -> p t b u", t=GT, b=B)
                pg = pgp.tile([128, GT * B], F32, tag="pg")
                if u > 0:
                    for t in range(GT):
                        for k in range(KB):
                            nc.tensor.matmul(
                                pg[:, t * B:(t + 1) * B],
                                whh_sb[k][:, t * 128:(t + 1) * 128],
                                hav[:, k, :, u - 1],
                                start=(k == 0), stop=(k == KB - 1))
                    nc.vector.tensor_tensor(
                        pg[:].rearrange("p (t b) -> p t b", t=GT),
                        pg[:].rearrange("p (t b) -> p t b", t=GT),
                        xpv[:, :, :, u], ALU.add)
                else:
                    nc.vector.tensor_copy(
                        pg[:].rearrange("p (t b) -> p t b", t=GT),
                        xpv[:, :, :, 0])
                ga = wp.tile([128, GT * B], F32, tag="ga")
                s = KB * B  # 8 cols per gate; i=[0:s], f=[s:2s], o=[2s:3s], g=[3s:4s]
                nc.scalar.activation(ga[:, 0:3 * s], pg[:, 0:3 * s],
                                     AF.Sigmoid, bias=0.0, scale=1.0)
                nc.scalar.activation(ga[:, 3 * s:4 * s], pg[:, 3 * s:4 * s],
                                     AF.Tanh, bias=0.0, scale=1.0)
                t2 = wp.tile([128, s], F32, tag="t2")
                nc.vector.tensor_tensor(t2[:], ga[:, 0:s], ga[:, 3 * s:4 * s],
                                        ALU.mult)
                t1 = wp.tile([128, s], F32, tag="t1")
                nc.vector.tensor_tensor(t1[:], ga[:, s:2 * s], cst[:], ALU.mult)
                nc.vector.tensor_tensor(cst[:], t1[:], t2[:], ALU.add)
                tch = wp.tile([128, s], F32, tag="tch")
                nc.scalar.activation(tch[:], cst[:], AF.Tanh, bias=0.0, scale=1.0)
                nc.vector.tensor_tensor(
                    hav[:, :, :, u], ga[:, 2 * s:3 * s].rearrange(
                        "p (k b) -> p k b", k=KB),
                    tch[:].rearrange("p (k b) -> p k b", k=KB), ALU.mult)

            def xp1_part(u0, t0, tn):
                # all t-windows in ONE psum bank; per-t biased drains
                xv = xp1[:].rearrange("p (t b u) -> p t b u", t=GT, b=B)
                hv = h0all[:].rearrange("p (k b u) -> p k b u", k=KB, b=B)
                s = B * UB
                pb = psp.tile([128, GT * s], F32, tag="ps")
                for t in range(t0, t0 + tn):
                    for k in range(KB):
                        nc.tensor.matmul(
                            pb[:, t * s:(t + 1) * s],
                            wih1_sb[k][:, t * 128:(t + 1) * 128],
                            hv[:, k, :, u0:u0 + UB],
                            start=(k == 0), stop=False)
                    # bias via K=1 matmul of b1 row against ones
                    nc.tensor.matmul(
                        pb[:, t * s:(t + 1) * s],
                        b1w_sb[0:1, t * 128:(t + 1) * 128], ones_sb[0:1, :],
                        start=False, stop=True)
                nc.vector.tensor_copy(
                    xv[:, :, :, u0:u0 + UB],
                    pb[:].rearrange("p (t b u) -> p t b u", t=GT, b=B))

            def dec_block(u0):
                dv = decp[:].rearrange("p (m b u) -> p m b u", m=KB, b=B)
                hv = h1all[:].rearrange("p (k b u) -> p k b u", k=KB, b=B)
                s = B * UB
                pb = psp.tile([128, GT * s], F32, tag="ps")
                for m in range(KB):
                    for k in range(KB):
                        nc.tensor.matmul(
                            pb[:, m * s:(m + 1) * s],
                            wdec_sb[k][:, m * 128:(m + 1) * 128],
                            hv[:, k, :, u0:u0 + UB],
                            start=(k == 0), stop=(k == KB - 1))
                nc.vector.tensor_copy(
                    dv[:, :, :, u0:u0 + UB],
                    pb[:, 0:KB * s].rearrange("p (m b u) -> p m b u", m=KB,
                                              b=B))

            # ---- joint ----
            units = {}  # (b, blk) -> dict(zt=..., obs=[...])

            def prep_preadd(b, g, ks):
                # zt[k,uo,:] = encp[k, b] + decp[k, b, g*8+uo]  (DVE 4x)
                un = units.setdefault((b, g), {})
                if "zt" not in un:
                    un["zt"] = zp.tile([128, KB * UB * T], BF16, tag="zt",
                                       name=f"zt_{b}_{g}")
                    un["obs"] = [obp.tile([128, UB * T], BF16, tag="ob",
                                          name=f"ob_{b}_{g}_{m}")
                                 for m in range(NM)]
                zt = un["zt"]
                for k in ks:
                    for uo in range(UB):
                        u = g * UB + uo
                        # SBUF-only op -> Pool (gpsimd); DVE/ACT stay free
                        # for PSUM drains (gpsimd may not touch PSUM)
                        nc.gpsimd.tensor_scalar_add(
                            zt[:, k * UB * T + uo * T:k * UB * T + (uo + 1) * T],
                            encp[:, k * RT + b * T:k * RT + (b + 1) * T],
                            decp[:, k * R + b * U + u:k * R + b * U + u + 1])

            def prep_tanh(b, g, ks):
                zt = units[(b, g)]["zt"]
                for k in ks:  # per-k so tanh trails the preadds closely
                    nc.scalar.activation(
                        zt[:, k * UB * T:(k + 1) * UB * T],
                        zt[:, k * UB * T:(k + 1) * UB * T],
                        AF.Tanh, bias=0.0, scale=1.0)

            def mm_duo(b, g, m, d):
                # pairs p=2d,2d+1 -> psum halves; drain on Pool w/ b_out
                un = units[(b, g)]
                zt, obs = un["zt"], un["obs"]
                mw = OMW[m]
                pj = POOLS["pj"].tile([128, 1024], F32, tag="pj")
                for h in range(2):
                    p = 2 * d + h
                    for k in range(KB):
                        nc.tensor.matmul(
                            pj[0:mw, h * 512:h * 512 + 2 * T],
                            wout_sb[k][:, m * 128:m * 128 + mw],
                            zt[:, k * UB * T + p * 2 * T:
                               k * UB * T + (p + 1) * 2 * T],
                            start=(k == 0), stop=(k == KB - 1))
                dst = obs[m][0:mw, d * 4 * T:(d + 1) * 4 * T].rearrange(
                    "p (s c) -> p s c", s=2)
                src = pj[0:mw, :].rearrange("p (s c) -> p s c", s=2)[:, :, 0:2 * T]
                # ~32 drains on ACT in tanh-free slots (b1 duos run i=0..3,
                # b0 duos i=4..7; tanh occupies ACT i=1..4), rest on DVE
                on_act = ((m, d) in ((0, 0), (1, 0)) if b == 1
                          else (m, d) in ((3, 0), (4, 0)))
                if on_act:
                    nc.scalar.activation(dst, src, AF.Identity,
                                         bias=bout_sb[0:mw, m:m + 1], scale=1.0)
                else:
                    nc.vector.tensor_scalar_add(dst, src, bout_sb[0:mw, m:m + 1])
                if d == 1:
                    nc.sync.dma_start(
                        out=outt[b, m * 128:m * 128 + mw,
                                 g * UB:(g + 1) * UB, :],
                        in_=obs[m][0:mw, :].rearrange("p (u t) -> p u t", u=UB))

            # duo issue order per unit: all 10 (m, d) pairs m-major
            DUOS = [(m, d) for m in range(NM) for d in range(2)]

            def joint_slot(blk, g, i):
                # mm duos issue FIRST each slot so dec/prep never block the
                # PE queue; dec+prep(b0) at i=1, tanh(b0) i=2 (consumed by
                # mm b0 from i=4), prep(b1) i=4, tanh(b1) i=6 (consumed
                # next blk i=0).
                if g >= 1:  # mm (b1, g-1): duos spread over i=0..3
                    lo, hi = [(0, 3), (3, 6), (6, 8), (8, 10)][i] if i < 4 \
                        else (0, 0)
                    for j in range(lo, hi):
                        mm_duo(1, g - 1, *DUOS[j])
                if g < NBLK and i >= 4:  # mm (b0, g): duos over i=4..7
                    lo, hi = [(0, 3), (3, 6), (6, 8), (8, 10)][i - 4]
                    for j in range(lo, hi):
                        mm_duo(0, g, *DUOS[j])
                if g < NBLK:
                    if i == 0:
                        dec_block(g * UB)
                        prep_preadd(0, g, (0, 1))
                    if i == 1:
                        prep_preadd(0, g, (2, 3))
                        prep_tanh(0, g, (0, 1))
                    if i == 2:
                        prep_preadd(1, g, (0, 1))
                        prep_tanh(0, g, (2, 3))
                    if i == 3:
                        prep_preadd(1, g, (2, 3))
                        prep_tanh(1, g, (0, 1))
                    if i == 4:
                        prep_tanh(1, g, (2, 3))

            with tc.tile_pool(name="ps0", bufs=2, space="PSUM") as ps0p:
                POOLS["ps0"] = ps0p
                in_proj(wih0_sb, lambda k: eyst_sb[k][:], b0_sb, xp0)
                for m in range(KB):
                    enc_proj(m)

            with tc.tile_pool(name="pj", bufs=3, space="PSUM") as pjp:
                POOLS["pj"] = pjp
                for blk in range(NBLK + 3):
                    g = blk - 2
                    for i in range(UB):
                        if blk >= 2:
                            joint_slot(blk, g, i)
                        if blk < NBLK:
                            lstm_step(blk * UB + i, xp0, whh0_sb, c0, h0all)
                        if 1 <= blk <= NBLK:
                            u0 = (blk - 1) * UB
                            if i == 0:
                                xp1_part(u0, 0, GT)
                            lstm_step(u0 + i, xp1, whh1_sb, c1, h1all)
    return nc


_CACHE = {}


def _prep_host(inputs):
    f32 = np.float32
    hs = np.asarray(inputs["hs_pad"], f32)
    ys = np.asarray(inputs["ys_in_pad"]).astype(np.int64)
    emb = np.asarray(inputs["embed_table"], f32)
    eys = emb[ys]  # (16, 64, 512)

    perm = np.concatenate([np.arange(0, 512), np.arange(512, 1024),
                           np.arange(1536, 2048), np.arange(1024, 1536)])

    def bt(x):  # transpose + bf16
        return np.ascontiguousarray(np.asarray(x, f32).T).astype(BF_NP)

    def btg(x):  # gate-permuted rows, then transpose + bf16
        return bt(np.asarray(x, f32)[perm])

    shared = {
        "wih0t": btg(inputs["W_ih0"]),
        "whh0t": btg(inputs["W_hh0"]),
        "wih1t": btg(inputs["W_ih1"]),
        "whh1t": btg(inputs["W_hh1"]),
        "wenct": bt(inputs["W_enc"]),
        "wdect": bt(inputs["W_dec"]),
        "woutt": bt(inputs["W_out"]),
        "bias0": np.ascontiguousarray(
            (np.asarray(inputs["b_ih0"], f32) + np.asarray(inputs["b_hh0"], f32))
            [perm].reshape(GT, 128).T),
        "b1w": np.ascontiguousarray(
            (np.asarray(inputs["b_ih1"], f32) + np.asarray(inputs["b_hh1"], f32))
            [perm].reshape(1, 4 * D)).astype(BF_NP),
        "benc": np.ascontiguousarray(
            np.asarray(inputs["b_enc"], f32).reshape(KB, 128).T),
    }
    bo = np.zeros(NM * 128, f32)
    bo[:ODIM] = np.asarray(inputs["b_out"], f32)
    shared["bout"] = np.ascontiguousarray(bo.reshape(NM, 128).T)

    in_maps = []
    for c in range(NCORES):
        m = dict(shared)
        m["hst"] = np.ascontiguousarray(
            hs[B * c:B * (c + 1)].reshape(RT, E).T).astype(BF_NP)
        m["eyst"] = np.ascontiguousarray(
            eys[B * c:B * (c + 1)].reshape(R, E).T).astype(BF_NP)
        in_maps.append(m)
    return in_maps


def kernel(**inputs):
    if "nc" not in _CACHE:
        nc_ = _build()
        if not nc_.is_finalized():
            nc_.finalize()
        _CACHE["nc"] = nc_
    nc = _CACHE["nc"]
    in_maps = _prep_host(inputs)
    trace = bool(int(os.environ.get("KERNEL_TRACE", "0")))
    res = run_bass_kernel_spmd(nc, in_maps, list(range(NCORES)), trace=trace)
    _CACHE["last"] = res
    out = np.empty((NCORES * B, T, U, ODIM), np.float32)
    for c in range(NCORES):
        oc = res.results[c]["outt"]  # (B, 600, 64, 200) bf16
        out[B * c:B * (c + 1)] = np.transpose(
            np.asarray(oc), (0, 3, 2, 1)).astype(np.float32)
    return out



# revision 5
# speedup vs baseline: 1.0600x; 1.0600x over previous
"""RNN-T decoder + joint network Trainium2 kernel (8 cores, data-parallel
over batch B=16 -> 2 per core; full inputs in, full output out).

Host side: embedding gather (indexing only), layout transposes, bf16 casts,
gate reorder i,f,g,o -> i,f,o,g, final bf16->fp32 upcast of the output.

Device side, per core, feature-major (partition = feature):
  * enc_pT = W_enc @ hsT + b_enc (bf16), xp0T = W_ih0 @ eysT + b0 as
    batched GEMMs (input projections hoisted out of the recurrence).
  * LSTM: W_hh matmuls -> PSUM gates; xp added by VectorE; sigmoid over
    i,f,o + tanh over g on ScalarE; state updates on VectorE.
  * Joint restructured for engine balance (CoreSim cost model):
      - DVE pre-add (4x mode, 112ns/200col): zt = encp_bf16 + decp[u]
      - ACT big-tile in-place tanh [128,1600] (1518ns for 8 u-cols)
      - PE W_out matmuls into 2-bank "duo" PSUM tiles [128,1024]
      - Pool (gpsimd) drains PSUM->obs bf16 with fused b_out (667ns/duo)
      - SP flushes obs [128,8u,200t] bf16 to DRAM (HWDGE)
    The joint is staggered one half-block: matmuls for (b1, blk-1) run
    while (b0, blk) pre-adds/tanhs, so PE never waits on prep.
  * Output DRAM tensor is bf16 (halves DMA_ENGINES time); host upcasts.
  * Input DMAs ordered so first compute (xp0: wih0+eyst; enc: wenc+hst)
    starts ~8us in instead of waiting for all weights.
"""

import os
import sys

import numpy as np

sys.path.insert(0, "/opt/trn_rl_repo")

import ml_dtypes  # noqa: E402
import concourse.bass as bass  # noqa: E402
from concourse import bacc  # noqa: E402
import concourse.mybir as mybir  # noqa: E402
import concourse.tile as tile  # noqa: E402
from concourse.bass_utils import run_bass_kernel_spmd  # noqa: E402

F32 = mybir.dt.float32
BF16 = mybir.dt.bfloat16
AF = mybir.ActivationFunctionType
ALU = mybir.AluOpType
BF_NP = ml_dtypes.bfloat16

NCORES = 8
B = 2        # batch per core
T = 200
U = 64
E = 512      # encoder proj dim
D = 512      # decoder hidden
J = 512      # joint dim
ODIM = 600
KB = 4       # 512 // 128
GT = 16      # 2048 // 128 gate tiles
R = B * U    # 128 LSTM rows per core
RT = B * T   # 400 encoder rows per core
OMW = [128, 128, 128, 128, 88]  # output feature tiles (600)
NM = len(OMW)
UB = 8       # u's per pipeline block
NBLK = U // UB


def _build():
    nc = bacc.Bacc()

    hst = nc.dram_tensor("hst", [E, RT], BF16, kind="ExternalInput")
    eyst = nc.dram_tensor("eyst", [E, R], BF16, kind="ExternalInput")
    wih0t = nc.dram_tensor("wih0t", [E, 4 * D], BF16, kind="ExternalInput")
    whh0t = nc.dram_tensor("whh0t", [D, 4 * D], BF16, kind="ExternalInput")
    wih1t = nc.dram_tensor("wih1t", [D, 4 * D], BF16, kind="ExternalInput")
    whh1t = nc.dram_tensor("whh1t", [D, 4 * D], BF16, kind="ExternalInput")
    wenct = nc.dram_tensor("wenct", [E, J], BF16, kind="ExternalInput")
    wdect = nc.dram_tensor("wdect", [D, J], BF16, kind="ExternalInput")
    woutt = nc.dram_tensor("woutt", [J, ODIM], BF16, kind="ExternalInput")
    bias0 = nc.dram_tensor("bias0", [128, GT], F32, kind="ExternalInput")
    b1w = nc.dram_tensor("b1w", [1, 4 * D], BF16, kind="ExternalInput")
    benc = nc.dram_tensor("benc", [128, KB], F32, kind="ExternalInput")
    bout = nc.dram_tensor("bout", [128, NM], F32, kind="ExternalInput")
    outt = nc.dram_tensor("outt", [B, ODIM, U, T], BF16, kind="ExternalOutput")

    with tile.TileContext(nc) as tc:
        with (
            tc.tile_pool(name="const", bufs=1) as cp,
            tc.tile_pool(name="work", bufs=2) as wp,
            tc.tile_pool(name="zt", bufs=5) as zp,
            tc.tile_pool(name="osb", bufs=11) as obp,
            tc.tile_pool(name="ps", bufs=1, space="PSUM") as psp,
            tc.tile_pool(name="pg", bufs=1, space="PSUM") as pgp,
        ):
            def load_kt(dram, cols, name):
                ts_ = []
                for k in range(dram.shape[0] // 128):
                    t = cp.tile([128, cols], BF16, tag=f"{name}{k}")
                    nc.sync.dma_start(out=t[:], in_=dram[k * 128:(k + 1) * 128, :])
                    ts_.append(t)
                return ts_

            def load_bias(dram, cols, name):
                raw = cp.tile([128, cols], F32, tag=f"{name}_raw")
                nc.sync.dma_start(out=raw[:], in_=dram[:, :])
                stg = cp.tile([128, cols], F32, tag=name)
                nc.vector.tensor_copy(stg[:], raw[:])  # stage onto DVE
                return stg

            # load order: first-needed first, k-tiles round-robin so the
            # k=0 accumulation chains can start after ~1/4 of the bytes
            # (xp0 <- wih0+eyst, enc <- wenc+hst, L0 <- whh0; wout last)
            def load_rr(specs):
                outs = [[] for _ in specs]
                for k in range(KB):
                    for si, (dram, cols, name) in enumerate(specs):
                        t = cp.tile([128, cols], BF16, tag=f"{name}{k}")
                        nc.sync.dma_start(
                            out=t[:], in_=dram[k * 128:(k + 1) * 128, :])
                        outs[si].append(t)
                return outs

            wih0_sb, eyst_sb, wenc_sb, hst_sb = load_rr([
                (wih0t, 4 * D, "wih0"), (eyst, R, "eyst"),
                (wenct, J, "wenc"), (hst, RT, "hst")])
            whh0_sb = load_kt(whh0t, 4 * D, "whh0")
            b0_sb = load_bias(bias0, GT, "b0")
            b1w_sb = cp.tile([1, 4 * D], BF16, tag="b1w")
            nc.sync.dma_start(out=b1w_sb[:], in_=b1w[:, :])
            ones_sb = cp.tile([1, B * UB], BF16, tag="ones")
            nc.vector.memset(ones_sb[:], 1.0)
            benc_sb = load_bias(benc, KB, "benc")
            bout_sb = load_bias(bout, NM, "bout")
            wih1_sb = load_kt(wih1t, 4 * D, "wih1")
            whh1_sb = load_kt(whh1t, 4 * D, "whh1")
            wdec_sb = load_kt(wdect, J, "wdec")
            wout_sb = load_kt(woutt, ODIM, "wout")

            # persistent state / intermediates
            # c01 col = l*8 + k*2 + b; h01all col = l*512 + k*128 + b*64 + u
            c01 = cp.tile([128, 2 * KB * B], F32, tag="c01")
            h01all = cp.tile([128, 2 * KB * R], BF16, tag="h01all")
            xp0 = cp.tile([128, GT * R], F32, tag="xp0")  # col t*128+b*64+u
            decp = cp.tile([128, KB * R], F32, tag="decp")  # col m*128+b*64+u
            encp = cp.tile([128, KB * RT], BF16, tag="encp")  # col m*400+b*200+t

            nc.vector.memset(c01[:], 0.0)

            POOLS = {}

            # ---- encoder projection: enc_pT = W_enc @ hsT + b_enc (bf16) ----
            def enc_proj(m):
                pe_ = POOLS["ps0"].tile([128, RT], F32, tag="ps")
                for k in range(KB):
                    nc.tensor.matmul(
                        pe_[:], wenc_sb[k][:, m * 128:(m + 1) * 128], hst_sb[k][:],
                        start=(k == 0), stop=(k == KB - 1))
                nc.vector.tensor_scalar_add(
                    encp[:, m * RT:(m + 1) * RT], pe_[:], benc_sb[:, m:m + 1])

            # ---- xp = W_ih @ rhs + bias (batched input projections) ----
            def in_proj(w_sb, rhs_fn, bias_sb, dst):
                for t in range(GT):
                    pb = POOLS["ps0"].tile([128, R], F32, tag="ps")
                    for k in range(KB):
                        nc.tensor.matmul(
                            pb[:], w_sb[k][:, t * 128:(t + 1) * 128], rhs_fn(k),
                            start=(k == 0), stop=(k == KB - 1))
                    nc.vector.tensor_scalar_add(
                        dst[:, t * R:(t + 1) * R], pb[:], bias_sb[:, t:t + 1])

            # gate order (host-permuted): i, f, o, g
            # merged lag-1 step s: layer0 computes u=s, layer1 computes u=s-1.
            # pg col = l*32 + t*2 + b (t = gate tile; i,f,o,g = t 0:4,4:8,8:12,12:16)
            def merged_step(s):
                l0 = s <= U - 1       # layer0 active
                l1 = 1 <= s <= U      # layer1 active
                hav = h01all[:].rearrange("p (l k b u) -> p l k b u", l=2, k=KB,
                                          b=B)
                pg = pgp.tile([128, 2 * GT * B], F32, tag="pg")
                pgv = pg[:].rearrange("p (l t b) -> p l t b", l=2, t=GT)
                if l0:
                    u = s
                    if u > 0:
                        for t in range(GT):
                            for k in range(KB):
                                nc.tensor.matmul(
                                    pgv[:, 0, t, :],
                                    whh0_sb[k][:, t * 128:(t + 1) * 128],
                                    hav[:, 0, k, :, u - 1],
                                    start=(k == 0), stop=(k == KB - 1))
                        nc.vector.tensor_tensor(
                            pgv[:, 0], pgv[:, 0],
                            xp0[:].rearrange("p (t b u) -> p t b u", t=GT,
                                             b=B)[:, :, :, u], ALU.add)
                    else:
                        nc.vector.tensor_copy(
                            pgv[:, 0],
                            xp0[:].rearrange("p (t b u) -> p t b u", t=GT,
                                             b=B)[:, :, :, 0])
                if l1:
                    u = s - 1
                    for t in range(GT):
                        for k in range(KB):
                            # wih1 @ h0[u] accumulated with whh1 @ h1[u-1]
                            nc.tensor.matmul(
                                pgv[:, 1, t, :],
                                wih1_sb[k][:, t * 128:(t + 1) * 128],
                                hav[:, 0, k, :, u],
                                start=(k == 0), stop=False)
                            if u > 0:
                                nc.tensor.matmul(
                                    pgv[:, 1, t, :],
                                    whh1_sb[k][:, t * 128:(t + 1) * 128],
                                    hav[:, 1, k, :, u - 1],
                                    start=False, stop=False)
                        nc.tensor.matmul(
                            pgv[:, 1, t, :],
                            b1w_sb[0:1, t * 128:(t + 1) * 128],
                            ones_sb[0:1, 0:B], start=False, stop=True)
                # activations: sigmoid over i,f,o; tanh over g (both layers)
                ls, lo = (0, 2) if (l0 and l1) else ((0, 1) if l0 else (1, 1))
                ga = wp.tile([128, 2 * GT * B], F32, tag="ga")
                gav = ga[:].rearrange("p (l t b) -> p l t b", l=2, t=GT)
                s3 = 3 * KB * B  # 24 cols per layer for i,f,o
                nc.scalar.activation(
                    gav[:, ls:ls + lo, 0:3 * KB, :], pgv[:, ls:ls + lo, 0:3 * KB, :],
                    AF.Sigmoid, bias=0.0, scale=1.0)
                nc.scalar.activation(
                    gav[:, ls:ls + lo, 3 * KB:, :], pgv[:, ls:ls + lo, 3 * KB:, :],
                    AF.Tanh, bias=0.0, scale=1.0)
                cv = c01[:].rearrange("p (l k b) -> p l k b", l=2, k=KB)
                i_g = gav[:, ls:ls + lo, 0:KB, :]
                f_g = gav[:, ls:ls + lo, KB:2 * KB, :]
                o_g = gav[:, ls:ls + lo, 2 * KB:3 * KB, :]
                g_g = gav[:, ls:ls + lo, 3 * KB:, :]
                cs = cv[:, ls:ls + lo]
                t2 = wp.tile([128, 2 * KB * B], F32, tag="t2")
                t2v = t2[:].rearrange("p (l k b) -> p l k b", l=2, k=KB)[:, ls:ls + lo]
                t1 = wp.tile([128, 2 * KB * B], F32, tag="t1")
                t1v = t1[:].rearrange("p (l k b) -> p l k b", l=2, k=KB)[:, ls:ls + lo]
                nc.vector.tensor_tensor(t2v, i_g, g_g, ALU.mult)
                nc.vector.tensor_tensor(t1v, f_g, cs, ALU.mult)
                nc.vector.tensor_tensor(cs, t1v, t2v, ALU.add)
                tch = wp.tile([128, 2 * KB * B], F32, tag="tch")
                tchv = tch[:].rearrange("p (l k b) -> p l k b", l=2, k=KB)[:, ls:ls + lo]
                nc.scalar.activation(tchv, cs, AF.Tanh, bias=0.0, scale=1.0)
                # h01 writes: layer0 at u=s, layer1 at u=s-1 (separate APs)
                if l0:
                    nc.vector.tensor_tensor(
                        hav[:, 0, :, :, s],
                        gav[:, 0, 2 * KB:3 * KB, :],
                        tch[:].rearrange("p (l k b) -> p l k b", l=2,
                                         k=KB)[:, 0], ALU.mult)
                if l1:
                    nc.vector.tensor_tensor(
                        hav[:, 1, :, :, s - 1],
                        gav[:, 1, 2 * KB:3 * KB, :],
                        tch[:].rearrange("p (l k b) -> p l k b", l=2,
                                         k=KB)[:, 1], ALU.mult)

            def dec_block(u0):
                dv = decp[:].rearrange("p (m b u) -> p m b u", m=KB, b=B)
                hv = h01all[:, KB * R:].rearrange("p (k b u) -> p k b u",
                                                  k=KB, b=B)
                s = B * UB
                pb = psp.tile([128, GT * s], F32, tag="ps")
                for m in range(KB):
                    for k in range(KB):
                        nc.tensor.matmul(
                            pb[:, m * s:(m + 1) * s],
                            wdec_sb[k][:, m * 128:(m + 1) * 128],
                            hv[:, k, :, u0:u0 + UB],
                            start=(k == 0), stop=(k == KB - 1))
                nc.vector.tensor_copy(
                    dv[:, :, :, u0:u0 + UB],
                    pb[:, 0:KB * s].rearrange("p (m b u) -> p m b u", m=KB,
                                              b=B))

            # ---- joint ----
            units = {}  # (b, blk) -> dict(zt=..., obs=[...])

            def prep_preadd(b, g, ks):
                # zt[k,uo,:] = encp[k, b] + decp[k, b, g*8+uo]  (DVE 4x)
                un = units.setdefault((b, g), {})
                if "zt" not in un:
                    un["zt"] = zp.tile([128, KB * UB * T], BF16, tag="zt",
                                       name=f"zt_{b}_{g}")
                    un["obs"] = [obp.tile([128, UB * T], BF16, tag="ob",
                                          name=f"ob_{b}_{g}_{m}")
                                 for m in range(NM)]
                zt = un["zt"]
                for k in ks:
                    for uo in range(UB):
                        u = g * UB + uo
                        # SBUF-only op -> Pool (gpsimd); DVE/ACT stay free
                        # for PSUM drains (gpsimd may not touch PSUM)
                        nc.gpsimd.tensor_scalar_add(
                            zt[:, k * UB * T + uo * T:k * UB * T + (uo + 1) * T],
                            encp[:, k * RT + b * T:k * RT + (b + 1) * T],
                            decp[:, k * R + b * U + u:k * R + b * U + u + 1])

            def prep_tanh(b, g, ks):
                zt = units[(b, g)]["zt"]
                for k in ks:  # per-k so tanh trails the preadds closely
                    nc.scalar.activation(
                        zt[:, k * UB * T:(k + 1) * UB * T],
                        zt[:, k * UB * T:(k + 1) * UB * T],
                        AF.Tanh, bias=0.0, scale=1.0)

            def mm_duo(b, g, m, d):
                # pairs p=2d,2d+1 -> psum halves; drain on Pool w/ b_out
                un = units[(b, g)]
                zt, obs = un["zt"], un["obs"]
                mw = OMW[m]
                pj = POOLS["pj"].tile([128, 1024], F32, tag="pj")
                for h in range(2):
                    p = 2 * d + h
                    for k in range(KB):
                        nc.tensor.matmul(
                            pj[0:mw, h * 512:h * 512 + 2 * T],
                            wout_sb[k][:, m * 128:m * 128 + mw],
                            zt[:, k * UB * T + p * 2 * T:
                               k * UB * T + (p + 1) * 2 * T],
                            start=(k == 0), stop=(k == KB - 1))
                dst = obs[m][0:mw, d * 4 * T:(d + 1) * 4 * T].rearrange(
                    "p (s c) -> p s c", s=2)
                src = pj[0:mw, :].rearrange("p (s c) -> p s c", s=2)[:, :, 0:2 * T]
                # ~32 drains on ACT in tanh-free slots (b1 duos run i=0..3,
                # b0 duos i=4..7; tanh occupies ACT i=1..4), rest on DVE
                on_act = ((m, d) in ((0, 0), (1, 0)) if b == 1
                          else (m, d) in ((3, 0), (4, 0)))
                if on_act:
                    nc.scalar.activation(dst, src, AF.Identity,
                                         bias=bout_sb[0:mw, m:m + 1], scale=1.0)
                else:
                    nc.vector.tensor_scalar_add(dst, src, bout_sb[0:mw, m:m + 1])
                if d == 1:
                    nc.sync.dma_start(
                        out=outt[b, m * 128:m * 128 + mw,
                                 g * UB:(g + 1) * UB, :],
                        in_=obs[m][0:mw, :].rearrange("p (u t) -> p u t", u=UB))

            # duo issue order per unit: all 10 (m, d) pairs m-major
            DUOS = [(m, d) for m in range(NM) for d in range(2)]

            def joint_slot(blk, g, i):
                # mm duos issue FIRST each slot so dec/prep never block the
                # PE queue; dec+prep(b0) at i=1, tanh(b0) i=2 (consumed by
                # mm b0 from i=4), prep(b1) i=4, tanh(b1) i=6 (consumed
                # next blk i=0).
                if g >= 1:  # mm (b1, g-1): duos spread over i=0..3
                    lo, hi = [(0, 3), (3, 6), (6, 8), (8, 10)][i] if i < 4 \
                        else (0, 0)
                    for j in range(lo, hi):
                        mm_duo(1, g - 1, *DUOS[j])
                if g < NBLK and i >= 4:  # mm (b0, g): duos over i=4..7
                    lo, hi = [(0, 3), (3, 6), (6, 8), (8, 10)][i - 4]
                    for j in range(lo, hi):
                        mm_duo(0, g, *DUOS[j])
                if g < NBLK:
                    if i == 0:
                        dec_block(g * UB)
                        prep_preadd(0, g, (0, 1))
                    if i == 1:
                        prep_preadd(0, g, (2, 3))
                        prep_tanh(0, g, (0, 1))
                    if i == 2:
                        prep_preadd(1, g, (0, 1))
                        prep_tanh(0, g, (2, 3))
                    if i == 3:
                        prep_preadd(1, g, (2, 3))
                        prep_tanh(1, g, (0, 1))
                    if i == 4:
                        prep_tanh(1, g, (2, 3))

            with tc.tile_pool(name="ps0", bufs=2, space="PSUM") as ps0p:
                POOLS["ps0"] = ps0p
                in_proj(wih0_sb, lambda k: eyst_sb[k][:], b0_sb, xp0)
                for m in range(KB):
                    enc_proj(m)

            with tc.tile_pool(name="pj", bufs=3, space="PSUM") as pjp:
                POOLS["pj"] = pjp
                for blk in range(NBLK + 3):
                    g = blk - 2
                    for i in range(UB):
                        if blk >= 2:
                            joint_slot(blk, g, i)
                        s = blk * UB + i
                        if s <= U:
                            merged_step(s)
    return nc


_CACHE = {}


def _prep_host(inputs):
    f32 = np.float32
    hs = np.asarray(inputs["hs_pad"], f32)
    ys = np.asarray(inputs["ys_in_pad"]).astype(np.int64)
    emb = np.asarray(inputs["embed_table"], f32)
    eys = emb[ys]  # (16, 64, 512)

    perm = np.concatenate([np.arange(0, 512), np.arange(512, 1024),
                           np.arange(1536, 2048), np.arange(1024, 1536)])

    def bt(x):  # transpose + bf16
        return np.ascontiguousarray(np.asarray(x, f32).T).astype(BF_NP)

    def btg(x):  # gate-permuted rows, then transpose + bf16
        return bt(np.asarray(x, f32)[perm])

    shared = {
        "wih0t": btg(inputs["W_ih0"]),
        "whh0t": btg(inputs["W_hh0"]),
        "wih1t": btg(inputs["W_ih1"]),
        "whh1t": btg(inputs["W_hh1"]),
        "wenct": bt(inputs["W_enc"]),
        "wdect": bt(inputs["W_dec"]),
        "woutt": bt(inputs["W_out"]),
        "bias0": np.ascontiguousarray(
            (np.asarray(inputs["b_ih0"], f32) + np.asarray(inputs["b_hh0"], f32))
            [perm].reshape(GT, 128).T),
        "b1w": np.ascontiguousarray(
            (np.asarray(inputs["b_ih1"], f32) + np.asarray(inputs["b_hh1"], f32))
            [perm].reshape(1, 4 * D)).astype(BF_NP),
        "benc": np.ascontiguousarray(
            np.asarray(inputs["b_enc"], f32).reshape(KB, 128).T),
    }
    bo = np.zeros(NM * 128, f32)
    bo[:ODIM] = np.asarray(inputs["b_out"], f32)
    shared["bout"] = np.ascontiguousarray(bo.reshape(NM, 128).T)

    in_maps = []
    for c in range(NCORES):
        m = dict(shared)
        m["hst"] = np.ascontiguousarray(
            hs[B * c:B * (c + 1)].reshape(RT, E).T).astype(BF_NP)
        m["eyst"] = np.ascontiguousarray(
            eys[B * c:B * (c + 1)].reshape(R, E).T).astype(BF_NP)
        in_maps.append(m)
    return in_maps


def kernel(**inputs):
    if "nc" not in _CACHE:
        nc_ = _build()
        if not nc_.is_finalized():
            nc_.finalize()
        _CACHE["nc"] = nc_
    nc = _CACHE["nc"]
    in_maps = _prep_host(inputs)
    trace = bool(int(os.environ.get("KERNEL_TRACE", "0")))
    res = run_bass_kernel_spmd(nc, in_maps, list(range(NCORES)), trace=trace)
    _CACHE["last"] = res
    out = np.empty((NCORES * B, T, U, ODIM), np.float32)
    for c in range(NCORES):
        oc = res.results[c]["outt"]  # (B, 600, 64, 200) bf16
        out[B * c:B * (c + 1)] = np.transpose(
            np.asarray(oc), (0, 3, 2, 1)).astype(np.float32)
    return out

